# revision 7
# baseline (speedup 1.0000x reference)
"""Distributed causal attention layer for 8 TRN2 NeuronCores.

Problem: payload [4, 2048, 1024], w_qkv [1024, 3072], w_out [1024, 1024],
b_out [1024] -> causal 16-head attention -> out [4, 2048, 1024], f32.

Sharding: core c handles batch b = c//2 and head-half half = c%2 (8 of 16
heads). Attention is fully local per (batch, head-half); the only cross-core
dependency is the output projection (contracts over all 16 heads), resolved
with pair-wise AllGathers of the normalized context (bf16, 4 x 512KB chunks
issued as each head-pair finishes so comm overlaps attention), after which
each core computes its 512 output columns.

Device dataflow (per core, bf16 matmuls, f32 PSUM accumulation):
  V [2048, 512] natural layout, ones-augmented per head (V' [128, 520]).
  Per head-pair hp: qT/kT [128, 2048] = w^T @ payloadT (head dim on
  partitions); then per head, k-major attention over q-halves:
  scoresT[k, q] = K @ Q^T (causal k-tiles only, triangular mask on diagonal
  blocks), one big exp per k-tile on ScalarE (scale=1/8 fused),
  ctx'T[65, q] = V'^T @ E accumulated over k (row 64 = softmax denominators
  from the ones column), normalization via bf16 K=1 broadcast matmul into
  the spare psum rows + reciprocal + multiply. Out-proj uses a K=1 ones
  matmul to add b_out. NOTE: an fp32 K=1 broadcast matmul variant hangs the
  PE (fp32 matmul lowering corner) — keep it bf16.
"""

import os
import numpy as np
import ml_dtypes

import concourse.bass as bass
import concourse.bacc as bacc
import concourse.mybir as mybir
import concourse.tile as tile
from concourse import bass_utils

B, S, D, H = 4, 2048, 1024, 16
DH = 64
HL = H // 2          # 8 local heads per core
DL = HL * DH         # 512 local head channels
N_CORES = 8
P = 128              # partitions
KC = D // P          # 8 contraction chunks for D
KT = S // P          # 16 k-tiles over sequence
QC = S // 512        # 4 q-chunks of 512
NEG = -1.0e9

BF16 = mybir.dt.bfloat16
F32 = mybir.dt.float32

LAST_EXEC_TIME_NS = None
_CACHED_NC = None


def _build():
    nc = bacc.Bacc(None, target_bir_lowering=False, debug=False)

    pt_d = nc.declare_dram_parameter("pt", [D, S], BF16, isOutput=False)
    wq_d = nc.declare_dram_parameter("wq", [D, DL], BF16, isOutput=False)
    wk_d = nc.declare_dram_parameter("wk", [D, DL], BF16, isOutput=False)
    wv_d = nc.declare_dram_parameter("wv", [D, DL], BF16, isOutput=False)
    wo_d = nc.declare_dram_parameter("wo", [D, DL], BF16, isOutput=False)
    bo_d = nc.declare_dram_parameter("bo", [1, DL], BF16, isOutput=False)
    out_d = nc.declare_dram_parameter("out", [S, DL], F32, isOutput=True)

    # upper-triangular causal mask block: tri[p, c] = 0 if p <= c else NEG
    tri_np = np.where(
        np.arange(P)[:, None] <= np.arange(P)[None, :], 0.0, NEG
    ).astype(np.float32)
    tri_d = nc.inline_tensor(tri_np, name="tri_const")

    with tile.TileContext(nc) as tc:
        with (
            tc.tile_pool(name="weights", bufs=1) as wpool,
            tc.tile_pool(name="payload", bufs=1) as ppool,
            tc.tile_pool(name="qk", bufs=1) as qkpool,
            tc.tile_pool(name="vp", bufs=1) as vpool,
            tc.tile_pool(name="ctxn", bufs=1) as cnpool,
            tc.tile_pool(name="ctxf", bufs=1) as cfpool,
            tc.tile_pool(name="et", bufs=3) as etpool,
            tc.tile_pool(name="stage", bufs=2) as stpool,
            tc.tile_pool(name="small", bufs=4) as smpool,
            tc.tile_pool(name="mmps", bufs=2, space="PSUM") as mmps,
            tc.tile_pool(name="scps", bufs=2, space="PSUM") as scps,
            tc.tile_pool(name="ctxps", bufs=2, space="PSUM") as ctxps,
            tc.tile_pool(name="dram", bufs=1, space="DRAM") as dpool,
        ):
            # ---- load inputs to SBUF ----
            pt_sb = []
            for kc in range(KC):
                t = ppool.tile([P, S], BF16, tag=f"pt{kc}", name="pt_sb")
                nc.sync.dma_start(out=t[:, :], in_=pt_d[kc * P:(kc + 1) * P, :])
                pt_sb.append(t)

            def load_w(dram, wname):
                tiles = []
                for kc in range(KC):
                    t = wpool.tile([P, DL], BF16, tag=f"{wname}{kc}", name="w_sb")
                    nc.sync.dma_start(
                        out=t[:, :], in_=dram[kc * P:(kc + 1) * P, :]
                    )
                    tiles.append(t)
                return tiles

            wq_sb = load_w(wq_d, "wq")
            wk_sb = load_w(wk_d, "wk")
            wv_sb = load_w(wv_d, "wv")
            wo_sb = load_w(wo_d, "wo")

            bo_sb = wpool.tile([1, DL], BF16, tag="bo")
            nc.sync.dma_start(out=bo_sb[:, :], in_=bo_d[:, :])
            tri_sb = wpool.tile([P, P], F32, tag="tri")
            nc.sync.dma_start(out=tri_sb[:, :], in_=tri_d[:, :])

            ones_bf = wpool.tile([1, P], BF16, tag="ones_bf")
            nc.vector.memset(ones_bf[:, :], 1.0)

            # ---- V' [128, 520] per k-tile: 8 heads x (64 V + ones col) ----
            vp_sb = []
            for st in range(KT):
                vt = vpool.tile([P, HL * (DH + 1)], BF16, tag=f"vp{st}",
                                name="vp_sb")
                ps = mmps.tile([P, DL], F32, tag="mm", name="ps")
                for kc in range(KC):
                    nc.tensor.matmul(
                        ps[:, :],
                        lhsT=pt_sb[kc][:, st * P:(st + 1) * P],
                        rhs=wv_sb[kc][:, :],
                        start=(kc == 0),
                        stop=(kc == KC - 1),
                    )
                nc.vector.tensor_copy(
                    out=vt.rearrange("p (h c) -> p h c", h=HL)[:, :, 0:DH],
                    in_=ps.rearrange("p (h c) -> p h c", h=HL)[:, :, :],
                )
                nc.vector.memset(
                    vt.rearrange("p (h c) -> p h c", h=HL)[:, :, DH:DH + 1], 1.0
                )
                vp_sb.append(vt)

            ctxn_sb = [cnpool.tile([P, S], BF16, tag=f"ctxn{i}", name="ctxn")
                       for i in range(DL // P)]
            ctxf_sb = [cfpool.tile([P, S], BF16, tag=f"ctxf{i}", name="ctxf_sb")
                       for i in range(KC)]
            cc_in = [dpool.tile([P, S], BF16, tag=f"ccin{i}", name="cc_in")
                     for i in range(DL // P)]
            cc_out = [dpool.tile([2 * P, S], BF16, tag=f"ccout{i}", name="cc_out")
                      for i in range(DL // P)]

            def norm(hp, pb, j, cps):
                # rows 0..63 = unnormalized ctxT, row 64 = denominators
                sums = smpool.tile([1, 512], BF16, tag="sums", name="sums")
                nc.vector.tensor_copy(out=sums[:, :], in_=cps[DH:DH + 1, :])
                # broadcast denominators into spare psum rows 64..127
                nc.tensor.matmul(
                    cps[DH:DH + DH, :], lhsT=ones_bf[0:1, 0:DH],
                    rhs=sums[0:1, :], start=True, stop=True,
                )
                rbc = smpool.tile([DH, 512], F32, tag="rbc", name="rbc")
                nc.vector.reciprocal(rbc[:, :], cps[DH:DH + DH, :])
                nc.vector.tensor_mul(
                    ctxn_sb[hp][pb:pb + DH, j * 512:(j + 1) * 512],
                    cps[0:DH, :],
                    rbc[:, :],
                )

            # ---- per head-pair: projections, attention, chunk AllGather ----
            for hp in range(DL // P):
                qt = kt = None
                for w_sb, qk_name in ((wq_sb, "q"), (wk_sb, "k")):
                    ot = qkpool.tile([P, S], BF16, tag=f"{qk_name}T{hp}",
                                     name="proj_sb")
                    for nj in range(QC):
                        ps = mmps.tile([P, 512], F32, tag="mm", name="ps")
                        for kc in range(KC):
                            nc.tensor.matmul(
                                ps[:, :],
                                lhsT=w_sb[kc][:, hp * P:(hp + 1) * P],
                                rhs=pt_sb[kc][:, nj * 512:(nj + 1) * 512],
                                start=(kc == 0),
                                stop=(kc == KC - 1),
                            )
                        nc.vector.tensor_copy(
                            out=ot[:, nj * 512:(nj + 1) * 512], in_=ps[:, :]
                        )
                    if qk_name == "q":
                        qt = ot
                    else:
                        kt = ot

                for hh in range(2):
                    h = hp * 2 + hh
                    pb = hh * DH
                    for qh in range(2):
                        jlist = [2 * qh, 2 * qh + 1]
                        cps_map = {
                            j: ctxps.tile([P, 512], F32, tag="ctx", name="cps")
                            for j in jlist
                        }
                        for t in range(8 * qh + 8):
                            valid_j = [j for j in jlist if 4 * j + 3 >= t]
                            sp = scps.tile([P, 1024], F32, tag="sc", name="sp")
                            for j in valid_j:
                                cb = (j - 2 * qh) * 512
                                nc.tensor.matmul(
                                    sp[:, cb:cb + 512],
                                    lhsT=kt[pb:pb + DH, t * P:(t + 1) * P],
                                    rhs=qt[pb:pb + DH, j * 512:(j + 1) * 512],
                                    start=True,
                                    stop=True,
                                )
                            jb = t // 4
                            et = etpool.tile([P, 1024], BF16, tag="et",
                                             name="et")
                            if jb in jlist:
                                # diagonal band k-tile
                                cb = (jb - 2 * qh) * 512
                                r = t % 4
                                nc.vector.tensor_add(
                                    sp[:, cb + r * P:cb + (r + 1) * P],
                                    sp[:, cb + r * P:cb + (r + 1) * P],
                                    tri_sb[:, :],
                                )
                                lo = cb + r * P
                                if r > 0:
                                    nc.vector.memset(et[:, cb:lo], 0.0)
                            else:
                                lo = 0
                            nc.scalar.activation(
                                et[:, lo:1024],
                                sp[:, lo:1024],
                                mybir.ActivationFunctionType.Exp,
                                scale=0.125,
                            )
                            for j in valid_j:
                                cb = (j - 2 * qh) * 512
                                nc.tensor.matmul(
                                    cps_map[j][0:DH + 1, :],
                                    lhsT=vp_sb[t][:, h * (DH + 1):
                                                  (h + 1) * (DH + 1)],
                                    rhs=et[:, cb:cb + 512],
                                    start=(t == 0),
                                    stop=(t == 4 * j + 3),
                                )
                                if t == 4 * j + 3:
                                    norm(hp, pb, j, cps_map[j])

                # chunk AllGather for this head-pair (overlaps next pair)
                nc.sync.dma_start(out=cc_in[hp][:, :], in_=ctxn_sb[hp][:, :])
                if os.environ.get("KERNEL_SKIP_CC") == "1":
                    nc.sync.dma_start(out=cc_out[hp][0:P, :], in_=cc_in[hp][:, :])
                    nc.sync.dma_start(out=cc_out[hp][P:2 * P, :],
                                      in_=cc_in[hp][:, :])
                else:
                    nc.gpsimd.collective_compute(
                        "AllGather",
                        mybir.AluOpType.bypass,
                        replica_groups=[[0, 1], [2, 3], [4, 5], [6, 7]],
                        ins=[cc_in[hp].opt()],
                        outs=[cc_out[hp].opt()],
                    )
                nc.sync.dma_start(out=ctxf_sb[2 * hp][:, :],
                                  in_=cc_out[hp][0:P, :])
                nc.sync.dma_start(out=ctxf_sb[2 * hp + 1][:, :],
                                  in_=cc_out[hp][P:2 * P, :])

            # ---- output projection (+ bias via K=1 ones matmul) ----
            # wo rows are host-permuted to match the gathered chunk order
            for mt in range(KT):
                ps = mmps.tile([P, DL], F32, tag="mm", name="ps")
                for kc in range(KC):
                    nc.tensor.matmul(
                        ps[:, :],
                        lhsT=ctxf_sb[kc][:, mt * P:(mt + 1) * P],
                        rhs=wo_sb[kc][:, :],
                        start=(kc == 0),
                        stop=False,
                    )
                nc.tensor.matmul(
                    ps[:, :], lhsT=ones_bf[0:1, :], rhs=bo_sb[0:1, :],
                    start=False, stop=True,
                )
                so = stpool.tile([P, DL], F32, tag="so", name="so")
                nc.scalar.copy(out=so[:, :], in_=ps[:, :])
                nc.sync.dma_start(
                    out=out_d[mt * P:(mt + 1) * P, :], in_=so[:, :]
                )

    nc.finalize()
    return nc


def kernel(payload, w_qkv, w_out, b_out):
    global LAST_EXEC_TIME_NS, _CACHED_NC
    payload = np.asarray(payload, dtype=np.float32)
    w_qkv = np.asarray(w_qkv, dtype=np.float32)
    w_out = np.asarray(w_out, dtype=np.float32)
    b_out = np.asarray(b_out, dtype=np.float32)

    bf = ml_dtypes.bfloat16
    # w_out rows permuted to match gathered ctx chunk order:
    # chunk 2*hp   = even core's head-pair hp -> rows [128hp, 128hp+128)
    # chunk 2*hp+1 = odd  core's head-pair hp -> rows [512+128hp, ...)
    row_perm = np.concatenate(
        [np.r_[128 * hp:128 * hp + 128, 512 + 128 * hp:512 + 128 * hp + 128]
         for hp in range(4)]
    )
    w_out_p = w_out[row_perm]

    in_maps = []
    for c in range(N_CORES):
        b, half = c // 2, c % 2
        cols = slice(half * DL, (half + 1) * DL)
        in_maps.append({
            "pt": np.ascontiguousarray(payload[b].T).astype(bf),
            "wq": np.ascontiguousarray(w_qkv[:, cols]).astype(bf),
            "wk": np.ascontiguousarray(w_qkv[:, D:][:, cols]).astype(bf),
            "wv": np.ascontiguousarray(w_qkv[:, 2 * D:][:, cols]).astype(bf),
            "wo": np.ascontiguousarray(w_out_p[:, cols]).astype(bf),
            "bo": np.ascontiguousarray(b_out[cols]).reshape(1, DL).astype(bf),
        })

    if _CACHED_NC is None:
        _CACHED_NC = _build()
    res = bass_utils.run_bass_kernel_spmd(
        _CACHED_NC, in_maps, core_ids=list(range(N_CORES))
    )
    LAST_EXEC_TIME_NS = res.exec_time_ns

    out = np.empty((B, S, D), dtype=np.float32)
    for c in range(N_CORES):
        b, half = c // 2, c % 2
        out[b, :, half * DL:(half + 1) * DL] = res.results[c]["out"]
    return out
